# revision 25
# baseline (speedup 1.0000x reference)
"""Trainium2 Bass kernel for a 4-layer GPT-2-style decoder (B=4, T=1024,
D=512, H=8, V=32000) with tied lm_head.

Sharding (8 cores): core c handles batch b = c//2 (body replicated across
the pair) and vocab half vh = c%2 of the lm_head. No collectives needed -
each core computes the full body for its batch, then logits[b, :, vh*16000:
(vh+1)*16000]. Host re-assembles the [4, 1024, 32000] output.

v2 (vs the v1 baseline at 1.67ms):
- LayerNorm: center-first (xc = x - mu), var from PE ones-matmul over xc^2,
  rstd = exp(-0.5*ln(var+eps)) so the whole LN + softmax pipeline stays in
  the natural_log_exp_and_others ACT table set (only gelu switches sets:
  2 switches/layer instead of ~4.5).
- LN gamma folded as xn = (xc*g)*rstd via one DVE STT per feature tile;
  LN beta folded into the consumer projection biases on the host.
- Attention: head pairs run with row-tiled concurrent K=64 score matmuls
  (partitions 0:64 / 64:128), one exp per (head, kt) spanning 2 PSUM banks,
  emission software-pipelined (scores of kt before AV of kt-1) so PE keeps
  streaming while ACT does exp. Softmax denominator comes from a ones
  column appended to V; per pair it is reciprocal'd ([2,T]), broadcast to
  128 partitions with one K=2 PE matmul, and applied with two DVE muls.
- MLP: m-outer loop loads Wh/Wmo once per layer (v1 loaded them twice),
  gelu hidden buffered bf16, second matmul in bf16 (Wmo bf16).
- Embedding/pos are uploaded pre-transposed (feature-major) so there are
  no on-device transposes.

Matmuls use float32r (full rate at N>=256); q/k/probs/v/h/Wmo use bf16.

Host-side input prep does only data movement + bias folding: sharding,
weight transposes/reshapes, dtype casts, and the embedding row lookup
(W_emb[input_ids] - pure indexing; this environment's runtime image has
no gpsimd ucode libraries and vector-offset DGE is disabled, so there is
no working device-side gather path).
"""

import os
import numpy as np
import ml_dtypes
from contextlib import ExitStack

import concourse.bass as bass
import concourse.tile as tile
from concourse import bacc, mybir
from concourse.bass_utils import run_bass_kernel_spmd

# Model dims (hardcoded per problem spec)
B, T, D, V, L, H = 4, 1024, 512, 32000, 4, 8
HD = D // H                 # 64
NF = D // 128               # 4 feature tiles
NTT = T // 128              # 8 token tiles
VH = V // 2                 # 16000 vocab half per core
NVC = 32                    # lm_head n-chunks
VC = VH // NVC              # 500 cols per chunk
LN_EPS = 1e-5
SCALE = 1.0 / np.sqrt(np.float32(D))

F32 = mybir.dt.float32
F32R = mybir.dt.float32r
BF16 = mybir.dt.bfloat16
AF = mybir.ActivationFunctionType
ALU = mybir.AluOpType

_CACHE = {}


def _r(ap):
    return ap.bitcast(F32R)


def _ln(nc, ones_sb, x_sb, xc_sb, xn_sb, g_col, stp, sqp, lnps, eps_col):
    """xn = (x - mu) * g * rstd, feature-major, chunk-pipelined.

    x_sb/xc_sb: [128, NF, T] fp32 SBUF; xn_sb may BE xc_sb (in-place STT) or
    a bf16 tile. g_col: [128, NF] per-partition scalars. Stats via
    ones-matmuls (M=128 -> broadcast across partitions). rstd =
    exp(-0.5*ln(var+eps)) keeps ACT in the nl_exp table set. All fp32
    outputs that feed fp32r matmuls are written through F32R-typed APs.
    """
    sum_ps = lnps.tile([128, T], F32, tag="lnsum")
    var_ps = lnps.tile([128, T], F32, tag="lnvar")
    xn_f32 = xn_sb.dtype == F32
    for c in range(2):
        cols = slice(c * 512, (c + 1) * 512)
        for f in range(NF):
            nc.tensor.matmul(sum_ps[:, cols], _r(ones_sb[:, 0:128]),
                             _r(x_sb[:, f, cols]),
                             start=(f == 0), stop=(f == NF - 1))
        mu = stp.tile([128, 512], F32, tag="mu")
        nc.vector.tensor_scalar_mul(mu[:], sum_ps[:, cols], 1.0 / D)
        for f in range(NF):
            nc.vector.tensor_sub(_r(xc_sb[:, f, cols]), x_sb[:, f, cols], mu[:])
        for f in range(NF):
            sq = sqp.tile([128, 512], F32, tag="sq")
            nc.scalar.square(_r(sq[:]), xc_sb[:, f, cols])
            nc.tensor.matmul(var_ps[:, cols], _r(ones_sb[:, 0:128]),
                             _r(sq[:]), start=(f == 0), stop=(f == NF - 1))
        lnv = stp.tile([128, 512], F32, tag="lnv")
        nc.scalar.activation(lnv[:], var_ps[:, cols], AF.Ln,
                             bias=eps_col[:], scale=1.0 / D)
        rstd = stp.tile([128, 512], F32, tag="rstd")
        nc.scalar.activation(rstd[:], lnv[:], AF.Exp, scale=-0.5)
        for f in range(NF):
            out = xn_sb[:, f, cols]
            nc.vector.scalar_tensor_tensor(_r(out) if xn_f32 else out,
                                           xc_sb[:, f, cols],
                                           g_col[:, f:f + 1], rstd[:],
                                           ALU.mult, ALU.mult)


def _build():
    nc = bacc.Bacc("TRN2", target_bir_lowering=False, debug=False)

    # ---- DRAM I/O ----
    embT = nc.dram_tensor("embT", [128, NF, T], F32, kind="ExternalInput").ap()
    posT = nc.dram_tensor("posT", [128, NF, T], F32, kind="ExternalInput").ap()
    wqkvT = nc.dram_tensor("wqkvT", [L, 128, NF, 3 * D], F32R, kind="ExternalInput").ap()
    woT = nc.dram_tensor("woT", [L, 128, NF, D], F32R, kind="ExternalInput").ap()
    whT = nc.dram_tensor("whT", [L, 128, NF, 4 * D], F32R, kind="ExternalInput").ap()
    wmoT = nc.dram_tensor("wmoT", [L, 16, 128, D], F32R, kind="ExternalInput").ap()
    lbias = nc.dram_tensor("lbias", [L, 128, 16], F32, kind="ExternalInput").ap()
    bh_sc = nc.dram_tensor("bh_sc", [L, 128, 16], F32, kind="ExternalInput").ap()
    bv_rows = nc.dram_tensor("bv_rows", [L, 1, D], F32R, kind="ExternalInput").ap()
    lng = nc.dram_tensor("lng", [L, 128, 8], F32, kind="ExternalInput").ap()
    lnf_p = nc.dram_tensor("lnf_p", [128, 8], F32, kind="ExternalInput").ap()
    mask_ut = nc.dram_tensor("mask_ut", [128, 128], BF16, kind="ExternalInput").ap()
    ones_in = nc.dram_tensor("ones_in", [128, 128], F32R, kind="ExternalInput").ap()
    whead = nc.dram_tensor("whead", [NVC, 128, NF, VC], F32R, kind="ExternalInput").ap()
    logits = nc.dram_tensor("logits", [NVC, T, VC], F32, kind="ExternalOutput").ap()
    DEBUG = bool(int(os.environ.get("KERNEL_DEBUG", "0")))
    dbg = {}
    if DEBUG:
        for nm in ("d_x0", "d_xn1", "d_z", "d_x1", "d_x2"):
            dbg[nm] = nc.dram_tensor(nm, [128, NF, T], F32, kind="ExternalOutput").ap()
        dbg["d_qk"] = nc.dram_tensor("d_qk", [128, 2 * NF, T], BF16, kind="ExternalOutput").ap()
        dbg["d_v"] = nc.dram_tensor("d_v", [128, NTT, H, 2 * HD], BF16, kind="ExternalOutput").ap()
        dbg["d_p00"] = nc.dram_tensor("d_p00", [128, T], BF16, kind="ExternalOutput").ap()
        dbg["d_p01"] = nc.dram_tensor("d_p01", [128, T], BF16, kind="ExternalOutput").ap()
        dbg["d_zg0"] = nc.dram_tensor("d_zg0", [128, T], F32, kind="ExternalOutput").ap()
        dbg["d_rdn0"] = nc.dram_tensor("d_rdn0", [64, T], F32, kind="ExternalOutput").ap()

    with tile.TileContext(nc) as tc, ExitStack() as ctx:
        const = ctx.enter_context(tc.tile_pool(name="const", bufs=1))
        ones_sb = const.tile([128, 128], F32R)
        nc.sync.dma_start(ones_sb[:], ones_in[:])
        mask_sb = const.tile([128, 128], BF16)
        nc.sync.dma_start(mask_sb[:], mask_ut[:])
        lnf_sb = const.tile([128, 8], F32)
        nc.sync.dma_start(lnf_sb[:], lnf_p[:])
        eps_col = const.tile([128, 1], F32)
        nc.vector.memset(eps_col[:], LN_EPS)

        xp = ctx.enter_context(tc.tile_pool(name="x", bufs=1))
        x_sb = xp.tile([128, NF, T], F32)
        xcp = ctx.enter_context(tc.tile_pool(name="xc", bufs=1))
        xc_sb = xcp.tile([128, NF, T], F32)
        wh0p = ctx.enter_context(tc.tile_pool(name="whead0", bufs=1))
        wch0 = wh0p.tile([128, NF, VC], F32R)
        nc.sync.dma_start(wch0[:], whead[0])
        vp2 = ctx.enter_context(tc.tile_pool(name="vaug", bufs=1))
        v_sb = vp2.tile([128, NTT, H, 2 * HD], BF16)
        nc.vector.memset(v_sb[:, :, :, HD:], 1.0)

        # ---- init: x = embT + posT (already feature-major) ----
        with tc.tile_pool(name="init", bufs=1) as initp:
            e_t = initp.tile([128, NF, T], F32)
            p_t = initp.tile([128, NF, T], F32)
            nc.sync.dma_start(e_t[:], embT[:])
            nc.sync.dma_start(p_t[:], posT[:])
            for c in range(2):
                cc = slice(c * 512, (c + 1) * 512)
                nc.vector.tensor_add(_r(x_sb[:, :, cc]), e_t[:, :, cc],
                                     p_t[:, :, cc])

        if DEBUG:
            nc.sync.dma_start(dbg["d_x0"][:], x_sb[:])

        # ---- layer pools (closed before lm_head) ----
        lctx = ctx.enter_context(ExitStack())
        stp = lctx.enter_context(tc.tile_pool(name="stat", bufs=2))
        sqp = lctx.enter_context(tc.tile_pool(name="sq", bufs=2))
        qkp = lctx.enter_context(tc.tile_pool(name="qk", bufs=1))
        zp = lctx.enter_context(tc.tile_pool(name="z", bufs=1))
        probp = lctx.enter_context(tc.tile_pool(name="probs", bufs=4))
        rdnp = lctx.enter_context(tc.tile_pool(name="rdn", bufs=1))
        hp = lctx.enter_context(tc.tile_pool(name="hsb", bufs=2))
        wq_p = lctx.enter_context(tc.tile_pool(name="wqkv", bufs=1))
        wo_p = lctx.enter_context(tc.tile_pool(name="wo", bufs=1))
        wh_p = lctx.enter_context(tc.tile_pool(name="wh", bufs=1))
        wmo_p = lctx.enter_context(tc.tile_pool(name="wmo", bufs=3))
        lb_p = lctx.enter_context(tc.tile_pool(name="lbias", bufs=1))

        for l in range(L):
            # layer weight loads (bufs=1: DMA overlaps prior-layer compute)
            wqkv_sb = wq_p.tile([128, NF, 3 * D], F32R, tag="wqkv")
            nc.sync.dma_start(wqkv_sb[:], wqkvT[l])
            wo_sb = wo_p.tile([128, NF, D], F32R, tag="wo")
            nc.sync.dma_start(wo_sb[:], woT[l])
            wh_sb = wh_p.tile([128, NF, 4 * D], F32R, tag="wh")
            nc.sync.dma_start(wh_sb[:], whT[l])
            lb_sb = lb_p.tile([128, 16], F32, tag="lb")
            nc.sync.dma_start(lb_sb[:], lbias[l])
            bh_sb = lb_p.tile([128, 16], F32, tag="bh")
            nc.sync.dma_start(bh_sb[:], bh_sc[l])
            bv_row = lb_p.tile([1, D], F32R, tag="bv")
            nc.sync.dma_start(bv_row[:], bv_rows[l])
            lng_sb = lb_p.tile([128, 8], F32, tag="lng")
            nc.sync.dma_start(lng_sb[:], lng[l])

            # ===== LN1 =====
            with tc.tile_pool(name="lnps1", bufs=1, space="PSUM") as lnps:
                _ln(nc, ones_sb, x_sb, xc_sb, xc_sb, lng_sb[:, 0:NF],
                    stp, sqp, lnps, eps_col)

            if DEBUG and l == 0:
                nc.sync.dma_start(dbg["d_xn1"][:], xc_sb[:])

            # ===== QKV =====
            qk_sb = qkp.tile([128, 2 * NF, T], BF16, tag="qk")
            with tc.tile_pool(name="qkvps", bufs=3, space="PSUM") as qkvps:
                # q (m 0..3) and k (m 4..7), feature-major
                for c in range(2):
                    cols = slice(c * 512, (c + 1) * 512)
                    for m in range(2 * NF):
                        ps = qkvps.tile([128, 512], F32, tag="qkv")
                        for kt in range(NF):
                            nc.tensor.matmul(
                                ps[:], wqkv_sb[:, kt, m * 128:(m + 1) * 128],
                                _r(xc_sb[:, kt, cols]),
                                start=(kt == 0), stop=(kt == NF - 1))
                        nc.scalar.activation(qk_sb[:, m, cols], ps[:],
                                             AF.Identity, bias=lb_sb[:, m:m + 1])
                # v, token-major (+bias via K=1 ones-row matmul)
                for tt in range(NTT):
                    trng = slice(tt * 128, (tt + 1) * 128)
                    ps = qkvps.tile([128, 512], F32, tag="qkv")
                    for kt in range(NF):
                        nc.tensor.matmul(ps[:], _r(xc_sb[:, kt, trng]),
                                         wqkv_sb[:, kt, 2 * D:3 * D],
                                         start=(kt == 0), stop=False)
                    nc.tensor.matmul(ps[:], _r(ones_sb[0:1, :]),
                                     bv_row[0:1, :], start=False, stop=True)
                    nc.vector.tensor_copy(
                        v_sb[:, tt, :, 0:HD],
                        ps[:].rearrange("p (h d) -> p h d", d=HD))

            if DEBUG and l == 0:
                nc.sync.dma_start(dbg["d_qk"][:], qk_sb[:])
                nc.sync.dma_start(dbg["d_v"][:], v_sb[:])

            # ===== attention (head pairs, pipelined scores/exp/AV) =====
            z_sb = zp.tile([128, NF, T], F32, tag="z")
            with tc.tile_pool(name="attnps", bufs=2, space="PSUM") as sps, \
                 tc.tile_pool(name="zps", bufs=2, space="PSUM") as zps:
                for hp_i in range(H // 2):
                    h0, h1 = 2 * hp_i, 2 * hp_i + 1
                    qblk, kblk = hp_i, NF + hp_i
                    zaug0 = zps.tile([128, T], F32, tag="zaug")
                    zaug1 = zps.tile([128, T], F32, tag="zaug")
                    zaug = {h0: zaug0, h1: zaug1}
                    pts = {}  # (h, kt) -> probs tile
                    for kt in range(NTT):
                        q0 = kt * 128
                        cols = T - q0
                        # scores + exp for both heads of the pair
                        for h in (h0, h1):
                            prow = (h % 2) * 64
                            ps = sps.tile([128, 1024], F32, tag="s")
                            for j in range(0, cols, 512):
                                n = min(512, cols - j)
                                nc.tensor.matmul(
                                    ps[:, j:j + n],
                                    qk_sb[prow:prow + 64, kblk, q0:q0 + 128],
                                    qk_sb[prow:prow + 64, qblk, q0 + j:q0 + j + n],
                                    start=True, stop=True)
                            pt = probp.tile([128, T], BF16, tag="p")
                            nc.scalar.activation(pt[:, 0:cols], ps[:, 0:cols],
                                                 AF.Exp, scale=float(SCALE))
                            nc.vector.tensor_mul(pt[:, 0:128], pt[:, 0:128],
                                                 mask_sb[:])
                            pts[(h, kt)] = pt
                            if DEBUG and l == 0 and hp_i == 0 and h == h0 and kt <= 1:
                                nc.sync.dma_start(dbg[f"d_p0{kt}"][:], pt[:])
                        # AV for kt-1 (software pipeline: PE stays ahead of ACT)
                        if kt > 0:
                            for h in (h0, h1):
                                _emit_av(nc, zaug[h], v_sb, pts.pop((h, kt - 1)),
                                         h, kt - 1)
                    for h in (h0, h1):
                        _emit_av(nc, zaug[h], v_sb, pts.pop((h, NTT - 1)),
                                 h, NTT - 1)
                    # per-head denominator: zaug rows 64:128 hold den
                    # (pre-broadcast by the ones block in V); recip to SBUF,
                    # then one TT mul per head.
                    for h, prow in ((h0, 0), (h1, 64)):
                        den_sb = rdnp.tile([64, T], F32, tag="den")
                        nc.scalar.copy(den_sb[:], zaug[h][HD:, :])
                        rdn = rdnp.tile([64, T], F32, tag="rdn")
                        nc.vector.reciprocal_approx_fast(out=rdn[:],
                                                         in_=den_sb[:])
                        nc.vector.tensor_mul(_r(z_sb[prow:prow + 64, qblk, :]),
                                             zaug[h][0:HD, :], rdn[:])
                        if DEBUG and l == 0 and h == 0:
                            zg = rdnp.tile([128, T], F32, tag="zgdbg")
                            nc.vector.tensor_copy(zg[:], zaug[h][:])
                            nc.sync.dma_start(dbg["d_zg0"][:], zg[:])
                            nc.sync.dma_start(dbg["d_rdn0"][:], rdn[:])

            if DEBUG and l == 0:
                nc.sync.dma_start(dbg["d_z"][:], z_sb[:])

            # ===== Wo projection + residual, then LN2 (chunk-interleaved) =====
            with tc.tile_pool(name="wops", bufs=2, space="PSUM") as wops:
                for c in range(2):
                    cols = slice(c * 512, (c + 1) * 512)
                    for f in range(NF):
                        ps = wops.tile([128, 512], F32, tag="wo")
                        for kt in range(NF):
                            nc.tensor.matmul(ps[:],
                                             wo_sb[:, kt, f * 128:(f + 1) * 128],
                                             _r(z_sb[:, kt, cols]),
                                             start=(kt == 0), stop=(kt == NF - 1))
                        nc.vector.scalar_tensor_tensor(
                            _r(x_sb[:, f, cols]), ps[:], lb_sb[:, 8 + f:9 + f],
                            x_sb[:, f, cols], ALU.add, ALU.add)

            if DEBUG and l == 0:
                nc.sync.dma_start(dbg["d_x1"][:], x_sb[:])

            # ===== LN2 (output straight to bf16: only the MLP consumes it) =====
            with tc.tile_pool(name="lnps2", bufs=1, space="PSUM") as lnps:
                _ln(nc, ones_sb, x_sb, xc_sb, xc_sb, lng_sb[:, NF:2 * NF],
                    stp, sqp, lnps, eps_col)

            # ===== MLP: h = gelu(Wh xn + bh); out = Wmo h (all fp32r) =====
            with tc.tile_pool(name="mlph", bufs=3, space="PSUM") as mlph, \
                 tc.tile_pool(name="mlpo", bufs=1, space="PSUM") as mlpo:
                for c in range(2):
                    cols = slice(c * 512, (c + 1) * 512)
                    ops = mlpo.tile([128, NF, 512], F32, tag="mo")
                    for m in range(16):
                        wmo_sb = wmo_p.tile([128, D], F32R, tag="wmo")
                        nc.sync.dma_start(wmo_sb[:], wmoT[l, m])
                        hps = mlph.tile([128, 512], F32, tag="mh")
                        for kt in range(NF):
                            nc.tensor.matmul(hps[:],
                                             wh_sb[:, kt, m * 128:(m + 1) * 128],
                                             _r(xc_sb[:, kt, cols]),
                                             start=(kt == 0), stop=(kt == NF - 1))
                        h_sb = hp.tile([128, 512], F32, tag="h")
                        nc.scalar.activation(_r(h_sb[:]), hps[:], AF.Gelu,
                                             bias=bh_sb[:, m:m + 1])
                        for f in range(NF):
                            nc.tensor.matmul(ops[:, f, :],
                                             wmo_sb[:, f * 128:(f + 1) * 128],
                                             _r(h_sb[:]),
                                             start=(m == 0), stop=(m == 15))
                    for f in range(NF):
                        nc.vector.scalar_tensor_tensor(
                            _r(x_sb[:, f, cols]), ops[:, f, :], lb_sb[:, 12 + f:13 + f],
                            x_sb[:, f, cols], ALU.add, ALU.add)

            if DEBUG and l == 0:
                nc.sync.dma_start(dbg["d_x2"][:], x_sb[:])

        # ===== final LN (affine, into x_sb in place) =====
        with tc.tile_pool(name="lnpsf", bufs=1, space="PSUM") as lnps:
            _ln(nc, ones_sb, x_sb, xc_sb, x_sb, lnf_sb[:, 0:NF],
                stp, sqp, lnps, eps_col)
        for f in range(NF):
            nc.vector.tensor_scalar_add(_r(x_sb[:, f, :]), x_sb[:, f, :],
                                        lnf_sb[:, NF + f:NF + f + 1])
        lctx.close()

        # ===== lm_head: logits[t, v] = xf.T @ whead =====
        with tc.tile_pool(name="whead", bufs=6) as whp, \
             tc.tile_pool(name="losb", bufs=8) as lop, \
             tc.tile_pool(name="hdps", bufs=6, space="PSUM") as hdps:
            for n in range(NVC):
                if n == 0:
                    wch = wch0
                else:
                    wch = whp.tile([128, NF, VC], F32R, tag="wch")
                    nc.sync.dma_start(wch[:], whead[n])
                for tt in range(NTT):
                    trng = slice(tt * 128, (tt + 1) * 128)
                    ps = hdps.tile([128, VC], F32, tag="hd")
                    for kt in range(NF):
                        nc.tensor.matmul(ps[:], _r(x_sb[:, kt, trng]),
                                         wch[:, kt, :],
                                         start=(kt == 0), stop=(kt == NF - 1))
                    o_sb = lop.tile([128, VC], F32, tag="lo")
                    if (n * NTT + tt) % 2 == 0:
                        nc.scalar.copy(o_sb[:], ps[:])
                    else:
                        nc.vector.tensor_copy(o_sb[:], ps[:])
                    nc.sync.dma_start(logits[n, trng, :], o_sb[:])

    nc.compile()
    return nc


def _emit_av(nc, zaug, v_sb, pt, h, kt):
    """zaug[:, q] += v_kt_aug.T @ probs, bank-aligned chunks with correct
    start/stop bookkeeping (zaug columns [512b, 512(b+1)) accumulate kt=0..
    min(4b+3, 7))."""
    q0 = kt * 128
    if kt < 4:
        chunks = [(q0, 512 - q0), (512, 512)]
    else:
        chunks = [(q0, T - q0)]
    for (qs, n) in chunks:
        b = qs // 512
        last_kt = min(4 * b + 3, NTT - 1)
        nc.tensor.matmul(zaug[:, qs:qs + n], v_sb[:, kt, h, :],
                         pt[:, qs - q0:qs - q0 + n],
                         start=(kt == 0), stop=(kt == last_kt))


def _to_sb(wt):
    """[K, O] -> [128, K//128, O] (partition-tiled along the contraction)."""
    k, o = wt.shape
    return np.ascontiguousarray(wt.reshape(k // 128, 128, o).swapaxes(0, 1))


def _feat_tiles(x):
    """[T, D] -> [128, D//128, T] feature-major (feature f*128+p at [p,f,t])."""
    return np.ascontiguousarray(x.T.reshape(-1, 128, x.shape[0]).swapaxes(0, 1))


def _col_sc(v):
    """[F] per-feature vector -> [128, F//128] per-partition scalar cols."""
    return np.ascontiguousarray(v.reshape(-1, 128).T)


def kernel(input_ids, W_emb, pos, Wqkv, bqkv, Wo, bo, ln1_g, ln1_b,
           ln2_g, ln2_b, Wh, bh, Wmo, bmo, lnf_g, lnf_b):
    input_ids = np.asarray(input_ids)
    W_emb = np.asarray(W_emb, dtype=np.float32)
    pos = np.asarray(pos, dtype=np.float32)
    Wqkv = np.asarray(Wqkv, dtype=np.float32)
    bqkv = np.asarray(bqkv, dtype=np.float32)
    Wo = np.asarray(Wo, dtype=np.float32)
    bo = np.asarray(bo, dtype=np.float32)
    ln1_g, ln1_b = np.asarray(ln1_g, np.float32), np.asarray(ln1_b, np.float32)
    ln2_g, ln2_b = np.asarray(ln2_g, np.float32), np.asarray(ln2_b, np.float32)
    Wh = np.asarray(Wh, dtype=np.float32)
    bh = np.asarray(bh, dtype=np.float32)
    Wmo = np.asarray(Wmo, dtype=np.float32)
    bmo = np.asarray(bmo, dtype=np.float32)
    lnf_g, lnf_b = np.asarray(lnf_g, np.float32), np.asarray(lnf_b, np.float32)

    if "nc" not in _CACHE:
        _CACHE["nc"] = _build()
    nc = _CACHE["nc"]

    # fold LN betas into the consumer projection biases (host-side)
    bqkv_f = np.stack([bqkv[l] + Wqkv[l] @ ln1_b[l] for l in range(L)])
    bh_f = np.stack([bh[l] + Wh[l] @ ln2_b[l] for l in range(L)])

    # shared (batch-independent) tensors
    shared = {}
    shared["posT"] = _feat_tiles(pos[:T])
    shared["wqkvT"] = np.stack([_to_sb(Wqkv[l].T) for l in range(L)])
    shared["woT"] = np.stack([_to_sb(Wo[l].T) for l in range(L)])
    shared["whT"] = np.stack([_to_sb(Wh[l].T) for l in range(L)])
    shared["wmoT"] = np.stack([_to_sb(Wmo[l].T).swapaxes(0, 1) for l in range(L)])
    shared["lbias"] = np.stack([
        np.concatenate([_col_sc(bqkv_f[l, 0:D]), _col_sc(bqkv_f[l, D:2 * D]),
                        _col_sc(bo[l]), _col_sc(bmo[l])], axis=1)
        for l in range(L)])
    shared["bh_sc"] = np.stack([_col_sc(bh_f[l]) for l in range(L)])
    shared["bv_rows"] = np.stack([bqkv_f[l, 2 * D:3 * D][None, :]
                                  for l in range(L)])
    shared["lng"] = np.stack([
        np.concatenate([_col_sc(ln1_g[l]), _col_sc(ln2_g[l])], axis=1)
        for l in range(L)])
    shared["lnf_p"] = np.concatenate([_col_sc(lnf_g), _col_sc(lnf_b)], axis=1)
    shared["mask_ut"] = np.triu(np.ones((128, 128))).astype(ml_dtypes.bfloat16)
    shared["ones_in"] = np.ones((128, 128), dtype=np.float32)
    wembT = W_emb.T  # [D, V]
    whead_halves = []
    for vh in range(2):
        wsb = _to_sb(wembT[:, vh * VH:(vh + 1) * VH])  # [128, NF, VH]
        wch = np.ascontiguousarray(
            wsb.reshape(128, NF, NVC, VC).transpose(2, 0, 1, 3))
        whead_halves.append(wch)

    in_maps = []
    for c in range(8):
        b, vh = c // 2, c % 2
        m = dict(shared)
        m["embT"] = _feat_tiles(W_emb[input_ids[b]])
        m["whead"] = whead_halves[vh]
        in_maps.append(m)

    res = run_bass_kernel_spmd(nc, in_maps, core_ids=list(range(8)),
                               trace=bool(int(os.environ.get("KERNEL_TRACE", "0"))))
    _CACHE["last_result"] = res

    out = np.empty((B, T, V), dtype=np.float32)
    for c in range(8):
        b, vh = c // 2, c % 2
        lg = res.results[c]["logits"]  # [NVC, T, VC]
        out[b, :, vh * VH:(vh + 1) * VH] = lg.transpose(1, 0, 2).reshape(T, VH)
    return out


# revision 26
# speedup vs baseline: 1.0835x; 1.0835x over previous
"""Trainium2 Bass kernel for a 4-layer GPT-2-style decoder (B=4, T=1024,
D=512, H=8, V=32000) with tied lm_head.

Sharding (8 cores): core c handles batch b = c//2 (body replicated across
the pair) and vocab half vh = c%2 of the lm_head. No collectives needed -
each core computes the full body for its batch, then logits[b, :, vh*16000:
(vh+1)*16000]. Host re-assembles the [4, 1024, 32000] output.

v2 (vs the v1 baseline at 1.67ms):
- LayerNorm: center-first (xc = x - mu), var from PE ones-matmul over xc^2,
  rstd = exp(-0.5*ln(var+eps)) so the whole LN + softmax pipeline stays in
  the natural_log_exp_and_others ACT table set (only gelu switches sets:
  2 switches/layer instead of ~4.5).
- LN gamma folded as xn = (xc*g)*rstd via one DVE STT per feature tile;
  LN beta folded into the consumer projection biases on the host.
- Attention: head pairs run with row-tiled concurrent K=64 score matmuls
  (partitions 0:64 / 64:128), one exp per (head, kt) spanning 2 PSUM banks,
  emission software-pipelined (scores of kt before AV of kt-1) so PE keeps
  streaming while ACT does exp. Softmax denominator comes from a ones
  column appended to V; per pair it is reciprocal'd ([2,T]), broadcast to
  128 partitions with one K=2 PE matmul, and applied with two DVE muls.
- MLP: m-outer loop loads Wh/Wmo once per layer (v1 loaded them twice),
  gelu hidden buffered bf16, second matmul in bf16 (Wmo bf16).
- Embedding/pos are uploaded pre-transposed (feature-major) so there are
  no on-device transposes.

Matmuls use float32r (full rate at N>=256); q/k/probs/v/h/Wmo use bf16.

Host-side input prep does only data movement + bias folding: sharding,
weight transposes/reshapes, dtype casts, and the embedding row lookup
(W_emb[input_ids] - pure indexing; this environment's runtime image has
no gpsimd ucode libraries and vector-offset DGE is disabled, so there is
no working device-side gather path).
"""

import os
import numpy as np
import ml_dtypes
from contextlib import ExitStack

import concourse.bass as bass
import concourse.tile as tile
from concourse import bacc, mybir
from concourse.bass_utils import run_bass_kernel_spmd

# Model dims (hardcoded per problem spec)
B, T, D, V, L, H = 4, 1024, 512, 32000, 4, 8
HD = D // H                 # 64
NF = D // 128               # 4 feature tiles
NTT = T // 128              # 8 token tiles
VH = V // 2                 # 16000 vocab half per core
NVC = 32                    # lm_head n-chunks
VC = VH // NVC              # 500 cols per chunk
LN_EPS = 1e-5
SCALE = 1.0 / np.sqrt(np.float32(D))

F32 = mybir.dt.float32
F32R = mybir.dt.float32r
BF16 = mybir.dt.bfloat16
AF = mybir.ActivationFunctionType
ALU = mybir.AluOpType

_CACHE = {}


def _r(ap):
    return ap.bitcast(F32R)


def _ln(nc, ones_sb, x_sb, xc_sb, xn_sb, g_col, stp, sqp, lnps, eps_col):
    """xn = (x - mu) * g * rstd, feature-major, chunk-pipelined.

    x_sb/xc_sb: [128, NF, T] fp32 SBUF; xn_sb may BE xc_sb (in-place STT) or
    a bf16 tile. g_col: [128, NF] per-partition scalars. Stats via
    ones-matmuls (M=128 -> broadcast across partitions). rstd =
    exp(-0.5*ln(var+eps)) keeps ACT in the nl_exp table set. All fp32
    outputs that feed fp32r matmuls are written through F32R-typed APs.
    """
    sum_ps = lnps.tile([128, T], F32, tag="lnsum")
    var_ps = lnps.tile([128, T], F32, tag="lnvar")
    xn_f32 = xn_sb.dtype == F32
    for c in range(2):
        cols = slice(c * 512, (c + 1) * 512)
        for f in range(NF):
            nc.tensor.matmul(sum_ps[:, cols], _r(ones_sb[:, 0:128]),
                             _r(x_sb[:, f, cols]),
                             start=(f == 0), stop=(f == NF - 1))
        mu = stp.tile([128, 512], F32, tag="mu")
        nc.vector.tensor_scalar_mul(mu[:], sum_ps[:, cols], 1.0 / D)
        for f in range(NF):
            nc.vector.tensor_sub(_r(xc_sb[:, f, cols]), x_sb[:, f, cols], mu[:])
        for f in range(NF):
            sq = sqp.tile([128, 512], F32, tag="sq")
            nc.scalar.square(_r(sq[:]), xc_sb[:, f, cols])
            nc.tensor.matmul(var_ps[:, cols], _r(ones_sb[:, 0:128]),
                             _r(sq[:]), start=(f == 0), stop=(f == NF - 1))
        lnv = stp.tile([128, 512], F32, tag="lnv")
        nc.scalar.activation(lnv[:], var_ps[:, cols], AF.Ln,
                             bias=eps_col[:], scale=1.0 / D)
        rstd = stp.tile([128, 512], F32, tag="rstd")
        nc.scalar.activation(rstd[:], lnv[:], AF.Exp, scale=-0.5)
        for f in range(NF):
            out = xn_sb[:, f, cols]
            nc.vector.scalar_tensor_tensor(_r(out) if xn_f32 else out,
                                           xc_sb[:, f, cols],
                                           g_col[:, f:f + 1], rstd[:],
                                           ALU.mult, ALU.mult)


def _build():
    nc = bacc.Bacc("TRN2", target_bir_lowering=False, debug=False)

    # ---- DRAM I/O ----
    embT = nc.dram_tensor("embT", [128, NF, T], F32, kind="ExternalInput").ap()
    posT = nc.dram_tensor("posT", [128, NF, T], F32, kind="ExternalInput").ap()
    wqkvT = nc.dram_tensor("wqkvT", [L, 128, NF, 3 * D], F32R, kind="ExternalInput").ap()
    woT = nc.dram_tensor("woT", [L, 128, NF, D], F32R, kind="ExternalInput").ap()
    whT = nc.dram_tensor("whT", [L, 128, NF, 4 * D], F32R, kind="ExternalInput").ap()
    wmoT = nc.dram_tensor("wmoT", [L, 16, 128, D], F32R, kind="ExternalInput").ap()
    lbias = nc.dram_tensor("lbias", [L, 128, 16], F32, kind="ExternalInput").ap()
    bh_sc = nc.dram_tensor("bh_sc", [L, 128, 16], F32, kind="ExternalInput").ap()
    bv_rows = nc.dram_tensor("bv_rows", [L, 1, D], F32R, kind="ExternalInput").ap()
    lng = nc.dram_tensor("lng", [L, 128, 8], F32, kind="ExternalInput").ap()
    lnf_p = nc.dram_tensor("lnf_p", [128, 8], F32, kind="ExternalInput").ap()
    mask_ut = nc.dram_tensor("mask_ut", [128, 128], BF16, kind="ExternalInput").ap()
    ones_in = nc.dram_tensor("ones_in", [128, 128], F32R, kind="ExternalInput").ap()
    whead = nc.dram_tensor("whead", [NVC, 128, NF, VC], F32R, kind="ExternalInput").ap()
    logits = nc.dram_tensor("logits", [NVC, T, VC], F32, kind="ExternalOutput").ap()
    DEBUG = bool(int(os.environ.get("KERNEL_DEBUG", "0")))
    dbg = {}
    if DEBUG:
        for nm in ("d_x0", "d_xn1", "d_z", "d_x1", "d_x2"):
            dbg[nm] = nc.dram_tensor(nm, [128, NF, T], F32, kind="ExternalOutput").ap()
        dbg["d_qk"] = nc.dram_tensor("d_qk", [128, 2 * NF, T], BF16, kind="ExternalOutput").ap()
        dbg["d_v"] = nc.dram_tensor("d_v", [128, NTT, H, 2 * HD], BF16, kind="ExternalOutput").ap()
        dbg["d_p00"] = nc.dram_tensor("d_p00", [128, T], BF16, kind="ExternalOutput").ap()
        dbg["d_p01"] = nc.dram_tensor("d_p01", [128, T], BF16, kind="ExternalOutput").ap()
        dbg["d_zg0"] = nc.dram_tensor("d_zg0", [128, T], F32, kind="ExternalOutput").ap()
        dbg["d_rdn0"] = nc.dram_tensor("d_rdn0", [64, T], F32, kind="ExternalOutput").ap()

    with tile.TileContext(nc) as tc, ExitStack() as ctx:
        const = ctx.enter_context(tc.tile_pool(name="const", bufs=1))
        ones_sb = const.tile([128, 128], F32R)
        nc.sync.dma_start(ones_sb[:], ones_in[:])
        mask_sb = const.tile([128, 128], BF16)
        nc.sync.dma_start(mask_sb[:], mask_ut[:])
        lnf_sb = const.tile([128, 8], F32)
        nc.sync.dma_start(lnf_sb[:], lnf_p[:])
        eps_col = const.tile([128, 1], F32)
        nc.vector.memset(eps_col[:], LN_EPS)

        xp = ctx.enter_context(tc.tile_pool(name="x", bufs=1))
        x_sb = xp.tile([128, NF, T], F32)
        xcp = ctx.enter_context(tc.tile_pool(name="xc", bufs=1))
        xc_sb = xcp.tile([128, NF, T], F32)
        wh0p = ctx.enter_context(tc.tile_pool(name="whead0", bufs=1))
        wch0 = wh0p.tile([128, NF, VC], F32R)
        vp2 = ctx.enter_context(tc.tile_pool(name="vaug", bufs=1))
        v_sb = vp2.tile([128, NTT, H, 2 * HD], BF16)
        nc.vector.memset(v_sb[:, :, :, HD:], 1.0)

        # ---- init: x = embT + posT (already feature-major) ----
        with tc.tile_pool(name="init", bufs=1) as initp:
            e_t = initp.tile([128, NF, T], F32)
            p_t = initp.tile([128, NF, T], F32)
            for c in range(2):
                cc = slice(c * 512, (c + 1) * 512)
                nc.sync.dma_start(e_t[:, :, cc], embT[:, :, cc])
                nc.sync.dma_start(p_t[:, :, cc], posT[:, :, cc])
            for c in range(2):
                cc = slice(c * 512, (c + 1) * 512)
                nc.vector.tensor_add(_r(x_sb[:, :, cc]), e_t[:, :, cc],
                                     p_t[:, :, cc])
            nc.sync.dma_start(wch0[:], whead[0])

        if DEBUG:
            nc.sync.dma_start(dbg["d_x0"][:], x_sb[:])

        # ---- layer pools (closed before lm_head) ----
        lctx = ctx.enter_context(ExitStack())
        stp = lctx.enter_context(tc.tile_pool(name="stat", bufs=2))
        sqp = lctx.enter_context(tc.tile_pool(name="sq", bufs=2))
        qkp = lctx.enter_context(tc.tile_pool(name="qk", bufs=1))
        zp = lctx.enter_context(tc.tile_pool(name="z", bufs=1))
        probp = lctx.enter_context(tc.tile_pool(name="probs", bufs=4))
        rdnp = lctx.enter_context(tc.tile_pool(name="rdn", bufs=1))
        hp = lctx.enter_context(tc.tile_pool(name="hsb", bufs=2))
        wq_p = lctx.enter_context(tc.tile_pool(name="wqkv", bufs=1))
        wo_p = lctx.enter_context(tc.tile_pool(name="wo", bufs=1))
        wh_p = lctx.enter_context(tc.tile_pool(name="wh", bufs=1))
        wmo_p = lctx.enter_context(tc.tile_pool(name="wmo", bufs=3))
        lb_p = lctx.enter_context(tc.tile_pool(name="lbias", bufs=1))

        for l in range(L):
            # layer weight loads (bufs=1: DMA overlaps prior-layer compute)
            wqkv_sb = wq_p.tile([128, NF, 3 * D], F32R, tag="wqkv")
            nc.sync.dma_start(wqkv_sb[:], wqkvT[l])
            wo_sb = wo_p.tile([128, NF, D], F32R, tag="wo")
            nc.sync.dma_start(wo_sb[:], woT[l])
            wh_sb = wh_p.tile([128, NF, 4 * D], F32R, tag="wh")
            nc.sync.dma_start(wh_sb[:], whT[l])
            lb_sb = lb_p.tile([128, 16], F32, tag="lb")
            nc.sync.dma_start(lb_sb[:], lbias[l])
            bh_sb = lb_p.tile([128, 16], F32, tag="bh")
            nc.sync.dma_start(bh_sb[:], bh_sc[l])
            bv_row = lb_p.tile([1, D], F32R, tag="bv")
            nc.sync.dma_start(bv_row[:], bv_rows[l])
            lng_sb = lb_p.tile([128, 8], F32, tag="lng")
            nc.sync.dma_start(lng_sb[:], lng[l])

            # ===== LN1 =====
            with tc.tile_pool(name="lnps1", bufs=1, space="PSUM") as lnps:
                _ln(nc, ones_sb, x_sb, xc_sb, xc_sb, lng_sb[:, 0:NF],
                    stp, sqp, lnps, eps_col)

            if DEBUG and l == 0:
                nc.sync.dma_start(dbg["d_xn1"][:], xc_sb[:])

            # ===== QKV =====
            qk_sb = qkp.tile([128, 2 * NF, T], BF16, tag="qk")
            with tc.tile_pool(name="qkvps", bufs=3, space="PSUM") as qkvps:
                # q (m 0..3) and k (m 4..7), feature-major
                for c in range(2):
                    cols = slice(c * 512, (c + 1) * 512)
                    for m in range(2 * NF):
                        ps = qkvps.tile([128, 512], F32, tag="qkv")
                        for kt in range(NF):
                            nc.tensor.matmul(
                                ps[:], wqkv_sb[:, kt, m * 128:(m + 1) * 128],
                                _r(xc_sb[:, kt, cols]),
                                start=(kt == 0), stop=(kt == NF - 1))
                        nc.scalar.activation(qk_sb[:, m, cols], ps[:],
                                             AF.Identity, bias=lb_sb[:, m:m + 1])
                # v, token-major (+bias via K=1 ones-row matmul)
                for tt in range(NTT):
                    trng = slice(tt * 128, (tt + 1) * 128)
                    ps = qkvps.tile([128, 512], F32, tag="qkv")
                    for kt in range(NF):
                        nc.tensor.matmul(ps[:], _r(xc_sb[:, kt, trng]),
                                         wqkv_sb[:, kt, 2 * D:3 * D],
                                         start=(kt == 0), stop=False)
                    nc.tensor.matmul(ps[:], _r(ones_sb[0:1, :]),
                                     bv_row[0:1, :], start=False, stop=True)
                    nc.vector.tensor_copy(
                        v_sb[:, tt, :, 0:HD],
                        ps[:].rearrange("p (h d) -> p h d", d=HD))

            if DEBUG and l == 0:
                nc.sync.dma_start(dbg["d_qk"][:], qk_sb[:])
                nc.sync.dma_start(dbg["d_v"][:], v_sb[:])

            # ===== attention (head pairs, pipelined scores/exp/AV) =====
            z_sb = zp.tile([128, NF, T], F32, tag="z")
            with tc.tile_pool(name="attnps", bufs=2, space="PSUM") as sps, \
                 tc.tile_pool(name="zps", bufs=2, space="PSUM") as zps:
                for hp_i in range(H // 2):
                    h0, h1 = 2 * hp_i, 2 * hp_i + 1
                    qblk, kblk = hp_i, NF + hp_i
                    zaug0 = zps.tile([128, T], F32, tag="zaug")
                    zaug1 = zps.tile([128, T], F32, tag="zaug")
                    zaug = {h0: zaug0, h1: zaug1}
                    pts = {}  # (h, kt) -> probs tile
                    for kt in range(NTT):
                        q0 = kt * 128
                        cols = T - q0
                        # scores + exp for both heads of the pair
                        for h in (h0, h1):
                            prow = (h % 2) * 64
                            ps = sps.tile([128, 1024], F32, tag="s")
                            for j in range(0, cols, 512):
                                n = min(512, cols - j)
                                nc.tensor.matmul(
                                    ps[:, j:j + n],
                                    qk_sb[prow:prow + 64, kblk, q0:q0 + 128],
                                    qk_sb[prow:prow + 64, qblk, q0 + j:q0 + j + n],
                                    start=True, stop=True)
                            pt = probp.tile([128, T], BF16, tag="p")
                            nc.scalar.activation(pt[:, 0:cols], ps[:, 0:cols],
                                                 AF.Exp, scale=float(SCALE))
                            nc.vector.tensor_mul(pt[:, 0:128], pt[:, 0:128],
                                                 mask_sb[:])
                            pts[(h, kt)] = pt
                            if DEBUG and l == 0 and hp_i == 0 and h == h0 and kt <= 1:
                                nc.sync.dma_start(dbg[f"d_p0{kt}"][:], pt[:])
                        # AV for kt-1 (software pipeline: PE stays ahead of ACT)
                        if kt > 0:
                            for h in (h0, h1):
                                _emit_av(nc, zaug[h], v_sb, pts.pop((h, kt - 1)),
                                         h, kt - 1)
                    for h in (h0, h1):
                        _emit_av(nc, zaug[h], v_sb, pts.pop((h, NTT - 1)),
                                 h, NTT - 1)
                    # per-head denominator: zaug rows 64:128 hold den
                    # (pre-broadcast by the ones block in V); recip to SBUF,
                    # then one TT mul per head.
                    for h, prow in ((h0, 0), (h1, 64)):
                        den_sb = rdnp.tile([64, T], F32, tag="den")
                        nc.scalar.copy(den_sb[:], zaug[h][HD:, :])
                        rdn = rdnp.tile([64, T], F32, tag="rdn")
                        nc.vector.reciprocal_approx_fast(out=rdn[:],
                                                         in_=den_sb[:])
                        nc.vector.tensor_mul(_r(z_sb[prow:prow + 64, qblk, :]),
                                             zaug[h][0:HD, :], rdn[:])
                        if DEBUG and l == 0 and h == 0:
                            zg = rdnp.tile([128, T], F32, tag="zgdbg")
                            nc.vector.tensor_copy(zg[:], zaug[h][:])
                            nc.sync.dma_start(dbg["d_zg0"][:], zg[:])
                            nc.sync.dma_start(dbg["d_rdn0"][:], rdn[:])

            if DEBUG and l == 0:
                nc.sync.dma_start(dbg["d_z"][:], z_sb[:])

            # ===== Wo projection + residual, then LN2 (chunk-interleaved) =====
            with tc.tile_pool(name="wops", bufs=2, space="PSUM") as wops:
                for c in range(2):
                    cols = slice(c * 512, (c + 1) * 512)
                    for f in range(NF):
                        ps = wops.tile([128, 512], F32, tag="wo")
                        for kt in range(NF):
                            nc.tensor.matmul(ps[:],
                                             wo_sb[:, kt, f * 128:(f + 1) * 128],
                                             _r(z_sb[:, kt, cols]),
                                             start=(kt == 0), stop=(kt == NF - 1))
                        nc.vector.scalar_tensor_tensor(
                            _r(x_sb[:, f, cols]), ps[:], lb_sb[:, 8 + f:9 + f],
                            x_sb[:, f, cols], ALU.add, ALU.add)

            if DEBUG and l == 0:
                nc.sync.dma_start(dbg["d_x1"][:], x_sb[:])

            # ===== LN2 (output straight to bf16: only the MLP consumes it) =====
            with tc.tile_pool(name="lnps2", bufs=1, space="PSUM") as lnps:
                _ln(nc, ones_sb, x_sb, xc_sb, xc_sb, lng_sb[:, NF:2 * NF],
                    stp, sqp, lnps, eps_col)

            # ===== MLP: h = gelu(Wh xn + bh); out = Wmo h (all fp32r) =====
            with tc.tile_pool(name="mlph", bufs=3, space="PSUM") as mlph, \
                 tc.tile_pool(name="mlpo", bufs=1, space="PSUM") as mlpo:
                for c in range(2):
                    cols = slice(c * 512, (c + 1) * 512)
                    ops = mlpo.tile([128, NF, 512], F32, tag="mo")
                    for m in range(16):
                        wmo_sb = wmo_p.tile([128, D], F32R, tag="wmo")
                        nc.sync.dma_start(wmo_sb[:], wmoT[l, m])
                        hps = mlph.tile([128, 512], F32, tag="mh")
                        for kt in range(NF):
                            nc.tensor.matmul(hps[:],
                                             wh_sb[:, kt, m * 128:(m + 1) * 128],
                                             _r(xc_sb[:, kt, cols]),
                                             start=(kt == 0), stop=(kt == NF - 1))
                        h_sb = hp.tile([128, 512], F32, tag="h")
                        nc.scalar.activation(_r(h_sb[:]), hps[:], AF.Gelu,
                                             bias=bh_sb[:, m:m + 1])
                        for f in range(NF):
                            nc.tensor.matmul(ops[:, f, :],
                                             wmo_sb[:, f * 128:(f + 1) * 128],
                                             _r(h_sb[:]),
                                             start=(m == 0), stop=(m == 15))
                    for f in range(NF):
                        nc.vector.scalar_tensor_tensor(
                            _r(x_sb[:, f, cols]), ops[:, f, :], lb_sb[:, 12 + f:13 + f],
                            x_sb[:, f, cols], ALU.add, ALU.add)

            if DEBUG and l == 0:
                nc.sync.dma_start(dbg["d_x2"][:], x_sb[:])

        # ===== final LN (affine, into x_sb in place) =====
        with tc.tile_pool(name="lnpsf", bufs=1, space="PSUM") as lnps:
            _ln(nc, ones_sb, x_sb, xc_sb, x_sb, lnf_sb[:, 0:NF],
                stp, sqp, lnps, eps_col)
        for f in range(NF):
            nc.vector.tensor_scalar_add(_r(x_sb[:, f, :]), x_sb[:, f, :],
                                        lnf_sb[:, NF + f:NF + f + 1])
        lctx.close()

        # ===== lm_head: logits[t, v] = xf.T @ whead =====
        with tc.tile_pool(name="whead", bufs=6) as whp, \
             tc.tile_pool(name="losb", bufs=2) as lop, \
             tc.tile_pool(name="hdps", bufs=6, space="PSUM") as hdps:
            for n in range(NVC):
                if n == 0:
                    wch = wch0
                else:
                    wch = whp.tile([128, NF, VC], F32R, tag="wch")
                    nc.sync.dma_start(wch[:], whead[n])
                o_sb = lop.tile([128, NTT, VC], F32, tag="lo")
                for tt in range(NTT):
                    trng = slice(tt * 128, (tt + 1) * 128)
                    ps = hdps.tile([128, VC], F32, tag="hd")
                    for kt in range(NF):
                        nc.tensor.matmul(ps[:], _r(x_sb[:, kt, trng]),
                                         wch[:, kt, :],
                                         start=(kt == 0), stop=(kt == NF - 1))
                    if tt % 2 == 0:
                        nc.scalar.copy(o_sb[:, tt, :], ps[:])
                    else:
                        nc.vector.tensor_copy(o_sb[:, tt, :], ps[:])
                nc.sync.dma_start(
                    logits[n].rearrange("(tt p) v -> p tt v", p=128), o_sb[:])

    nc.compile()
    return nc


def _emit_av(nc, zaug, v_sb, pt, h, kt):
    """zaug[:, q] += v_kt_aug.T @ probs, bank-aligned chunks with correct
    start/stop bookkeeping (zaug columns [512b, 512(b+1)) accumulate kt=0..
    min(4b+3, 7))."""
    q0 = kt * 128
    if kt < 4:
        chunks = [(q0, 512 - q0), (512, 512)]
    else:
        chunks = [(q0, T - q0)]
    for (qs, n) in chunks:
        b = qs // 512
        last_kt = min(4 * b + 3, NTT - 1)
        nc.tensor.matmul(zaug[:, qs:qs + n], v_sb[:, kt, h, :],
                         pt[:, qs - q0:qs - q0 + n],
                         start=(kt == 0), stop=(kt == last_kt))


def _to_sb(wt):
    """[K, O] -> [128, K//128, O] (partition-tiled along the contraction)."""
    k, o = wt.shape
    return np.ascontiguousarray(wt.reshape(k // 128, 128, o).swapaxes(0, 1))


def _feat_tiles(x):
    """[T, D] -> [128, D//128, T] feature-major (feature f*128+p at [p,f,t])."""
    return np.ascontiguousarray(x.T.reshape(-1, 128, x.shape[0]).swapaxes(0, 1))


def _col_sc(v):
    """[F] per-feature vector -> [128, F//128] per-partition scalar cols."""
    return np.ascontiguousarray(v.reshape(-1, 128).T)


def kernel(input_ids, W_emb, pos, Wqkv, bqkv, Wo, bo, ln1_g, ln1_b,
           ln2_g, ln2_b, Wh, bh, Wmo, bmo, lnf_g, lnf_b):
    input_ids = np.asarray(input_ids)
    W_emb = np.asarray(W_emb, dtype=np.float32)
    pos = np.asarray(pos, dtype=np.float32)
    Wqkv = np.asarray(Wqkv, dtype=np.float32)
    bqkv = np.asarray(bqkv, dtype=np.float32)
    Wo = np.asarray(Wo, dtype=np.float32)
    bo = np.asarray(bo, dtype=np.float32)
    ln1_g, ln1_b = np.asarray(ln1_g, np.float32), np.asarray(ln1_b, np.float32)
    ln2_g, ln2_b = np.asarray(ln2_g, np.float32), np.asarray(ln2_b, np.float32)
    Wh = np.asarray(Wh, dtype=np.float32)
    bh = np.asarray(bh, dtype=np.float32)
    Wmo = np.asarray(Wmo, dtype=np.float32)
    bmo = np.asarray(bmo, dtype=np.float32)
    lnf_g, lnf_b = np.asarray(lnf_g, np.float32), np.asarray(lnf_b, np.float32)

    if "nc" not in _CACHE:
        _CACHE["nc"] = _build()
    nc = _CACHE["nc"]

    # fold LN betas into the consumer projection biases (host-side)
    bqkv_f = np.stack([bqkv[l] + Wqkv[l] @ ln1_b[l] for l in range(L)])
    bh_f = np.stack([bh[l] + Wh[l] @ ln2_b[l] for l in range(L)])

    # shared (batch-independent) tensors
    shared = {}
    shared["posT"] = _feat_tiles(pos[:T])
    shared["wqkvT"] = np.stack([_to_sb(Wqkv[l].T) for l in range(L)])
    shared["woT"] = np.stack([_to_sb(Wo[l].T) for l in range(L)])
    shared["whT"] = np.stack([_to_sb(Wh[l].T) for l in range(L)])
    shared["wmoT"] = np.stack([_to_sb(Wmo[l].T).swapaxes(0, 1) for l in range(L)])
    shared["lbias"] = np.stack([
        np.concatenate([_col_sc(bqkv_f[l, 0:D]), _col_sc(bqkv_f[l, D:2 * D]),
                        _col_sc(bo[l]), _col_sc(bmo[l])], axis=1)
        for l in range(L)])
    shared["bh_sc"] = np.stack([_col_sc(bh_f[l]) for l in range(L)])
    shared["bv_rows"] = np.stack([bqkv_f[l, 2 * D:3 * D][None, :]
                                  for l in range(L)])
    shared["lng"] = np.stack([
        np.concatenate([_col_sc(ln1_g[l]), _col_sc(ln2_g[l])], axis=1)
        for l in range(L)])
    shared["lnf_p"] = np.concatenate([_col_sc(lnf_g), _col_sc(lnf_b)], axis=1)
    shared["mask_ut"] = np.triu(np.ones((128, 128))).astype(ml_dtypes.bfloat16)
    shared["ones_in"] = np.ones((128, 128), dtype=np.float32)
    wembT = W_emb.T  # [D, V]
    whead_halves = []
    for vh in range(2):
        wsb = _to_sb(wembT[:, vh * VH:(vh + 1) * VH])  # [128, NF, VH]
        wch = np.ascontiguousarray(
            wsb.reshape(128, NF, NVC, VC).transpose(2, 0, 1, 3))
        whead_halves.append(wch)

    in_maps = []
    for c in range(8):
        b, vh = c // 2, c % 2
        m = dict(shared)
        m["embT"] = _feat_tiles(W_emb[input_ids[b]])
        m["whead"] = whead_halves[vh]
        in_maps.append(m)

    res = run_bass_kernel_spmd(nc, in_maps, core_ids=list(range(8)),
                               trace=bool(int(os.environ.get("KERNEL_TRACE", "0"))))
    _CACHE["last_result"] = res

    out = np.empty((B, T, V), dtype=np.float32)
    for c in range(8):
        b, vh = c // 2, c % 2
        lg = res.results[c]["logits"]  # [NVC, T, VC]
        out[b, :, vh * VH:(vh + 1) * VH] = lg.transpose(1, 0, 2).reshape(T, VH)
    return out
